# revision 2
# baseline (speedup 1.0000x reference)
"""MoE inverted-dispatch expert bank kernel for 8x Trainium2 NeuronCores.

Strategy (expert parallelism + load packing):
  - Host: replicate the reference routing (stable argsort -> per-expert rank,
    capacity drop), build a transposed per-expert token buffer bufT[E, d, C],
    cast weights + tokens to bf16.
  - Experts are sorted by token count and dealt round-robin into 8 "slots" x
    8 cores, so every core runs the same per-slot capacity schedule (SPMD:
    one program, 8 cores) while matmuls only cover the tokens actually
    routed (padded per slot band), roughly halving PE work vs CAP=256.
  - Device (per core): per expert slot, grouped 2-layer MLP with weights as
    the stationary matmul operand and the token buffer moving in
    [feature, token] layout:  hT[f, C] = gelu(W1-tiles.T @ bufT),
    yT[d, C] = W2-tiles @ hT.  fp32 PSUM accumulation, GELU on ScalarE.
  - Host: gather yT, scatter-combine to [N, k, d], loads = counts / N.
"""

import os
import numpy as np
import ml_dtypes

N_TOK = 4096
K_ACT = 2
D_MODEL = 512
D_FF = 2048
N_EXP = 64
CAP = 256
N_CORES = 8
E_PER = N_EXP // N_CORES  # 8 expert slots per core

P = 128
KT1 = D_MODEL // P   # 4  k-tiles for layer 1
MT1 = D_FF // P      # 16 m-tiles for layer 1
KT2 = D_FF // P      # 16 k-tiles for layer 2
MT2 = D_MODEL // P   # 4  m-tiles for layer 2

LAST_EXEC_TIME_NS = None

_CACHE = {}


def _install_trace_hook():
    try:
        from antenv.axon_hooks import get_axon_ntff_profile_hook  # noqa: F401
        return True
    except ImportError:
        pass
    try:
        import sys, types
        import trn_agent_boot.trn_boot as tb
        hook = tb._ntff_profile_via_ctypes('/opt/axon/libaxon_pjrt.so')
        mod = types.ModuleType("antenv.axon_hooks")
        mod.get_axon_ntff_profile_hook = lambda: hook
        mod.set_axon_ntff_profile_hook = lambda h: None
        sys.modules['antenv.axon_hooks'] = mod
        import antenv
        antenv.axon_hooks = mod
        return True
    except Exception:
        return False


def _build_kernel(caps):
    """caps: per-slot moving widths (tokens per expert slot), len E_PER."""
    import concourse.tile as tile
    import concourse.mybir as mybir
    from concourse import bacc
    from concourse.bass import ts, ds

    bf16 = mybir.dt.bfloat16
    f32 = mybir.dt.float32

    nc = bacc.Bacc("TRN2", target_bir_lowering=False, debug=False)
    bufT = nc.dram_tensor("bufT", [E_PER, D_MODEL, CAP], bf16, kind="ExternalInput")
    w1 = nc.dram_tensor("w1", [E_PER, D_MODEL, D_FF], bf16, kind="ExternalInput")
    w2 = nc.dram_tensor("w2", [E_PER, D_FF, D_MODEL], bf16, kind="ExternalInput")
    yT = nc.dram_tensor("yT", [E_PER, D_MODEL, CAP], bf16, kind="ExternalOutput")

    with tile.TileContext(nc) as tc:
        with (
            tc.tile_pool(name="xpool", bufs=3) as xpool,
            tc.tile_pool(name="w1pool", bufs=12) as w1pool,
            tc.tile_pool(name="w2pool", bufs=8) as w2pool,
            tc.tile_pool(name="hpool", bufs=3) as hpool,
            tc.tile_pool(name="ypool", bufs=4) as ypool,
            tc.tile_pool(name="ps1", bufs=4, space="PSUM") as ps1pool,
            tc.tile_pool(name="ps2", bufs=4, space="PSUM") as ps2pool,
        ):
            for e in range(E_PER):
                C = caps[e]
                # layer-1 weight k-slabs: [128 (d rows), 2048 (f)]
                w1s = []
                for k in range(KT1):
                    w1t = w1pool.tile([P, D_FF], bf16, tag="w1")
                    nc.sync.dma_start(w1t[:], w1[e, ts(k, P), :])
                    w1s.append(w1t)
                xt = xpool.tile([P, KT1, C], bf16, tag="x")
                nc.sync.dma_start(
                    xt[:], bufT[e].rearrange("(ko p) c -> p ko c", p=P)[:, :, :C]
                )

                ht = hpool.tile([P, KT2, C], bf16, tag="h")
                w2s = []
                for m in range(MT1):
                    if m % 4 == 0:
                        # prefetch layer-2 weight quarter-slab during layer 1
                        q = m // 4
                        w2t = w2pool.tile([P, 4, D_MODEL], bf16, tag="w2")
                        nc.sync.dma_start(
                            w2t[:],
                            w2[e, ds(q * 512, 512), :].rearrange(
                                "(ko p) d -> p ko d", p=P
                            ),
                        )
                        w2s.append(w2t)
                    ps = ps1pool.tile([P, C], f32, tag="ps1")
                    for k in range(KT1):
                        nc.tensor.matmul(
                            ps[:],
                            lhsT=w1s[k][:, ts(m, P)],
                            rhs=xt[:, k, :],
                            start=(k == 0),
                            stop=(k == KT1 - 1),
                        )
                    nc.scalar.activation(
                        ht[:, m, :], ps[:], mybir.ActivationFunctionType.Gelu
                    )

                for m in range(MT2):
                    ps = ps2pool.tile([P, C], f32, tag="ps2")
                    for k in range(KT2):
                        nc.tensor.matmul(
                            ps[:],
                            lhsT=w2s[k // 4][:, k % 4, ts(m, P)],
                            rhs=ht[:, k, :],
                            start=(k == 0),
                            stop=(k == KT2 - 1),
                        )
                    yt = ypool.tile([P, C], bf16, tag="y")
                    nc.vector.tensor_copy(yt[:], ps[:])
                    nc.sync.dma_start(yT[e, ts(m, P), :C], yt[:])

    nc.compile()
    return nc


def kernel(hidden_states, selected_experts, expert_masks, W1, W2):
    global LAST_EXEC_TIME_NS
    from concourse.bass_utils import run_bass_kernel_spmd

    hidden = np.ascontiguousarray(np.asarray(hidden_states, dtype=np.float32))
    sel = np.asarray(selected_experts).astype(np.int64)
    W1 = np.asarray(W1, dtype=np.float32)
    W2 = np.asarray(W2, dtype=np.float32)

    # ---- host dispatch (mirrors reference routing exactly) ----
    flat_e = sel.reshape(-1)
    S = flat_e.shape[0]
    order = np.argsort(flat_e, kind="stable")
    e_sorted = flat_e[order]
    counts = np.bincount(flat_e, minlength=N_EXP)
    offsets = np.cumsum(counts) - counts
    rank = np.arange(S, dtype=np.int64) - offsets[e_sorted]
    tok = order // K_ACT
    slot = order % K_ACT
    valid = rank < CAP
    ev, rv, tv, sv = e_sorted[valid], rank[valid], tok[valid], slot[valid]

    bf = ml_dtypes.bfloat16
    hidden16 = hidden.astype(bf)
    bufT = np.zeros((N_EXP, D_MODEL, CAP), dtype=bf)
    bufT[ev, :, rv] = hidden16[tv]
    W1_16 = W1.astype(bf)
    W2_16 = W2.astype(bf)

    # ---- pack: sort experts by load, deal bands of 8 across the 8 cores ----
    counts_used = np.minimum(counts, CAP)
    perm = np.argsort(-counts_used, kind="stable")  # experts, busiest first
    perm_js = perm.reshape(E_PER, N_CORES)  # [slot j, core i] -> expert id
    caps = tuple(
        int(np.ceil(max(int(counts_used[perm_js[j]].max()), 16) / 16) * 16)
        for j in range(E_PER)
    )

    trace = os.environ.get("KERNEL_TRACE", "0") == "1" and _install_trace_hook()

    if caps not in _CACHE:
        _CACHE[caps] = _build_kernel(caps)
    nc = _CACHE[caps]

    in_maps = [
        {
            "bufT": np.ascontiguousarray(bufT[perm_js[:, i]]),
            "w1": np.ascontiguousarray(W1_16[perm_js[:, i]]),
            "w2": np.ascontiguousarray(W2_16[perm_js[:, i]]),
        }
        for i in range(N_CORES)
    ]

    res = run_bass_kernel_spmd(
        nc, in_maps, core_ids=list(range(N_CORES)), trace=trace
    )
    LAST_EXEC_TIME_NS = res.exec_time_ns

    yT = np.zeros((N_EXP, D_MODEL, CAP), dtype=np.float32)
    for i in range(N_CORES):
        yT[perm_js[:, i]] = res.results[i]["yT"].astype(np.float32)

    # ---- host combine ----
    gathered = yT[ev, :, rv]  # [Sv, D_MODEL]
    expert_outputs = np.zeros((N_TOK, K_ACT, D_MODEL), dtype=np.float32)
    expert_outputs[tv, sv] = gathered
    expert_loads = counts.astype(np.float32) / N_TOK
    return expert_outputs, expert_loads
